# revision 1
# baseline (speedup 1.0000x reference)
"""Weighted histogram-binning kernel for Trainium2 (8 NeuronCores).

Problem: simmat [64,4,32,4096] f32, mask [64,32,4096] bool ->
hist [64,4,32,29] f32 where hist[b,c,q,n] = sum_d mask[b,q,d] *
(int((simmat[b,c,q,d]+1.00001)/2*28) == n).

Strategy: pure data parallelism over the batch dim (8 batches/core).
Per core, 8 tiles of [128 rows = (c,q), 4096] are processed.

Binning is via cumulative counts cnt_lt(t) = #unmasked{y < t}, t=1..29,
y = (x + 1.00001) * 14 (two fp32 roundings, bit-identical to the
reference's (x+1.00001)/2*28 since the /2 is exact). hist = adjacent
differences (host side, exact integer arithmetic).

v1: 29 fused DVE scalar_tensor_tensor ops/tile ((y<t)*w + row-sum).
v3: integer bins are computed once via r = int-convert(y) plus the
    rounding-mode-agnostic fixup floor = r - (r > y), then mask-merged
    to bm = (floor-100)*w in bf16 (unmasked: exact integers in
    [-100,-72]; masked: 0, above every shifted threshold t-100).
    The 29 thresholds are split across engines, one fused op each per
    tile (measured ~4.3-4.4us per [128,4096] op on HW; accum_out
    forces 1x rate on DVE, so DVE and ACT ops cost about the same):
      DVE:  tensor_scalar is_lt + accum row-sum      (13 thresholds)
      ACT:  Sign(bm + (100.5-t)) + accum row-sum     (16 thresholds,
            cnt_lt = (N - S)/2, masked elements cancel)
    GPSIMD runs the convert, subtract and mask-merge prep ops.
"""
import numpy as np
from contextlib import ExitStack

import concourse.bass as bass
import concourse.tile as tile
from concourse import bacc, mybir
from concourse.bass_utils import run_bass_kernel_spmd

B, C, Q, D = 64, 4, 32, 4096
NCORES = 8
BPC = B // NCORES          # batches per core
ROWS = C * Q               # 128 rows per batch -> one SBUF tile
NB = 29
NT = 32                    # padded threshold columns in the output

AOT = mybir.AluOpType
AFT = mybir.ActivationFunctionType
F32 = mybir.dt.float32
BF16 = mybir.dt.bfloat16

# thresholds t=1..29; these go to the scalar (ACT) engine, rest to DVE.
# HW measurement: tensor_scalar with accum_out runs at 1x (~4.4us/op on
# [128,4096]), same as one ACT Sign+accum op (~4.25us) -> split nearly
# evenly, ACT slightly favored since DVE also carries the prep ops.
ACT_T = list(range(14, 30))
SPLIT_T = 29               # this ACT threshold's columns are split ACT/DVE
SPLIT_D = 2048             # ACT takes columns [0, SPLIT_D), DVE the rest

_nc_cache = {}
last_results = None  # test.py reads exec info from here
TRACE = False

STRATEGY = "v3"


def _build_v1(nc, x_ap, w_ap, o_ap, c_ap, tc, ctx):
    xp = ctx.enter_context(tc.tile_pool(name="xp", bufs=2))
    wp = ctx.enter_context(tc.tile_pool(name="wp", bufs=2))
    sp = ctx.enter_context(tc.tile_pool(name="sp", bufs=2))
    op = ctx.enter_context(tc.tile_pool(name="op", bufs=2))
    for t in range(BPC):
        xt = xp.tile([ROWS, D], F32)
        nc.sync.dma_start(xt[:], x_ap[t])
        wt = wp.tile([ROWS, D], F32)
        for c in range(C):
            nc.sync.dma_start(wt[c * Q:(c + 1) * Q, :], w_ap[t])
        y = sp.tile([ROWS, D], F32, tag="y")
        nc.vector.tensor_scalar(y[:], xt[:], 1.00001, 14.0, AOT.add, AOT.mult)
        scr = sp.tile([ROWS, D], F32, tag="scr")
        cnt = op.tile([ROWS, NT], F32)
        for n in range(1, NB + 1):
            nc.vector.scalar_tensor_tensor(
                scr[:], y[:], float(n), wt[:], AOT.is_lt, AOT.mult,
                accum_out=cnt[:, n - 1:n])
        nc.sync.dma_start(o_ap[t, :, 0:NT], cnt[:])


def _build_v3(nc, x_ap, w_ap, o_ap, c_ap, tc, ctx):
    dve_t = [t for t in range(1, NB + 1) if t not in ACT_T]
    cp = ctx.enter_context(tc.tile_pool(name="cp", bufs=1))
    bias = cp.tile([ROWS, NT], F32)
    nc.sync.dma_start(bias[:], c_ap[:])

    xp = ctx.enter_context(tc.tile_pool(name="xp", bufs=2))
    wp = ctx.enter_context(tc.tile_pool(name="wp", bufs=2))
    yp = ctx.enter_context(tc.tile_pool(name="yp", bufs=2))
    ip = ctx.enter_context(tc.tile_pool(name="ip", bufs=2))
    bp = ctx.enter_context(tc.tile_pool(name="bp", bufs=2))
    gp = ctx.enter_context(tc.tile_pool(name="gp", bufs=2))
    mp = ctx.enter_context(tc.tile_pool(name="mp", bufs=2))
    sp = ctx.enter_context(tc.tile_pool(name="sp", bufs=1))
    op = ctx.enter_context(tc.tile_pool(name="op", bufs=2))

    for t in range(BPC):
        xt = xp.tile([ROWS, D], F32)
        nc.sync.dma_start(xt[:], x_ap[t])
        wt = wp.tile([ROWS, D], BF16)
        for c in range(C):
            nc.sync.dma_start(wt[c * Q:(c + 1) * Q, :], w_ap[t])

        # y = (x + 1.00001) * 14       [DVE, fp32 2x]
        y = yp.tile([ROWS, D], F32, tag="y")
        nc.vector.tensor_scalar(y[:], xt[:], 1.00001, 14.0, AOT.add, AOT.mult)
        # r = int(y) (i32; RNE on DVE, rounding mode irrelevant: the
        # gt-fixup below corrects any bi in {floor, floor+1})  [GPSIMD]
        bi = ip.tile([ROWS, D], mybir.dt.int32, tag="bi")
        nc.gpsimd.tensor_copy(bi[:], y[:])
        # b100 = r - 100 -> bf16       [GPSIMD, i32 in]
        b100 = bp.tile([ROWS, D], BF16, tag="b100")
        nc.gpsimd.tensor_scalar(b100[:], bi[:], 100.0, None, AOT.subtract)
        # gt = (r > y) -> bf16         [DVE TT 1x]
        gt = gp.tile([ROWS, D], BF16, tag="gt")
        nc.vector.tensor_tensor(gt[:], bi[:], y[:], AOT.is_gt)
        # bm0 = floor(y)-100 = b100-gt [GPSIMD bf16]
        bm0 = mp.tile([ROWS, D], BF16, tag="bm0")
        nc.gpsimd.tensor_tensor(bm0[:], b100[:], gt[:], AOT.subtract)
        # bm = bm0 * w                 [GPSIMD bf16]
        bm = mp.tile([ROWS, D], BF16, tag="bm")
        nc.gpsimd.tensor_tensor(bm[:], bm0[:], wt[:], AOT.mult)

        cnt = op.tile([ROWS, NT], F32, tag="cnt")
        scr_d = sp.tile([ROWS, D], BF16, tag="scr_d")
        scr_a = sp.tile([ROWS, D], BF16, tag="scr_a")
        # interleave DVE and ACT issue so both engines fill early
        order = []
        di, ai = 0, 0
        while di < len(dve_t) or ai < len(ACT_T):
            if di < len(dve_t):
                order.append(("d", dve_t[di])); di += 1
            if ai < len(ACT_T):
                order.append(("a", ACT_T[ai])); ai += 1
        for kind, n in order:
            if kind == "d":
                # cnt[:, n-1] = sum_d (bm < n-100)          [DVE 1x]
                nc.vector.tensor_scalar(scr_d[:], bm[:], float(n - 100), None,
                                        AOT.is_lt, AOT.add,
                                        accum_out=cnt[:, n - 1:n])
            elif n == SPLIT_T:
                # ACT is the critical path by ~1 op-half: split this
                # threshold's columns between ACT (first SPLIT_D) and
                # DVE (rest); host adds the two partial counts.
                nc.scalar.activation(scr_a[:, 0:SPLIT_D], bm[:, 0:SPLIT_D],
                                     AFT.Sign, bias=bias[:, n - 1:n],
                                     accum_out=cnt[:, n - 1:n])
                nc.vector.tensor_scalar(scr_d[:, SPLIT_D:D], bm[:, SPLIT_D:D],
                                        float(n - 100), None,
                                        AOT.is_lt, AOT.add,
                                        accum_out=cnt[:, NB:NB + 1])
            else:
                # cnt[:, n-1] = sum_d sign(bm + (100.5-n))  [ACT]
                nc.scalar.activation(scr_a[:], bm[:], AFT.Sign,
                                     bias=bias[:, n - 1:n],
                                     accum_out=cnt[:, n - 1:n])
        nc.sync.dma_start(o_ap[t, :, 0:NT], cnt[:])


def _build(strategy=None):
    strategy = strategy or STRATEGY
    if strategy in _nc_cache:
        return _nc_cache[strategy]
    nc = bacc.Bacc("TRN2", target_bir_lowering=False, debug=False,
                   enable_asserts=False, num_devices=NCORES)
    x_ap = nc.dram_tensor("x", [BPC, ROWS, D], F32, kind="ExternalInput").ap()
    w_dt = F32 if strategy == "v1" else BF16
    w_ap = nc.dram_tensor("w", [BPC, Q, D], w_dt, kind="ExternalInput").ap()
    c_ap = nc.dram_tensor("consts", [ROWS, NT], F32, kind="ExternalInput").ap()
    o_ap = nc.dram_tensor("o", [BPC, ROWS, NT], F32, kind="ExternalOutput").ap()

    with tile.TileContext(nc) as tc, ExitStack() as ctx:
        if strategy == "v1":
            _build_v1(nc, x_ap, w_ap, o_ap, c_ap, tc, ctx)
        elif strategy == "v3":
            _build_v3(nc, x_ap, w_ap, o_ap, c_ap, tc, ctx)
        else:
            raise ValueError(strategy)
    nc.compile()
    _nc_cache[strategy] = nc
    return nc


def _make_consts():
    consts = np.zeros((ROWS, NT), np.float32)
    for n in ACT_T:
        consts[:, n - 1] = 100.5 - n
    return consts


def kernel(simmat: np.ndarray, mask: np.ndarray) -> np.ndarray:
    global last_results
    strategy = STRATEGY
    nc = _build(strategy)
    consts = _make_consts()
    in_maps = []
    import ml_dtypes
    w_np = np.float32 if strategy == "v1" else ml_dtypes.bfloat16
    for ci in range(NCORES):
        sl = slice(ci * BPC, (ci + 1) * BPC)
        xs = np.ascontiguousarray(
            np.asarray(simmat[sl], dtype=np.float32).reshape(BPC, ROWS, D))
        ws = np.ascontiguousarray(np.asarray(mask[sl]).astype(w_np))
        in_maps.append({"x": xs, "w": ws, "consts": consts})
    res = run_bass_kernel_spmd(nc, in_maps, core_ids=list(range(NCORES)),
                               trace=TRACE)
    last_results = res
    raw = np.concatenate([r["o"] for r in res.results], axis=0)
    raw = raw.reshape(B * C * Q, NT)

    # reconstruct cnt_lt(t), t=1..29 per row
    cnt = np.empty((B * C * Q, NB), np.float64)
    if strategy == "v1":
        cnt[:] = raw[:, :NB]
    else:
        for t in range(1, NB + 1):
            if t == SPLIT_T:
                cnt[:, t - 1] = ((SPLIT_D - raw[:, t - 1]) * 0.5
                                 + raw[:, NB])
            elif t in ACT_T:
                cnt[:, t - 1] = (D - raw[:, t - 1]) * 0.5
            else:
                cnt[:, t - 1] = raw[:, t - 1]
    hist = np.diff(cnt, prepend=0.0, axis=-1)
    return np.ascontiguousarray(
        hist.reshape(B, C, Q, NB).astype(np.float32))



# revision 3
# speedup vs baseline: 1.5359x; 1.5359x over previous
"""Weighted histogram-binning kernel for Trainium2 (8 NeuronCores).

Problem: simmat [64,4,32,4096] f32, mask [64,32,4096] bool ->
hist [64,4,32,29] f32 where hist[b,c,q,n] = sum_d mask[b,q,d] *
(int((simmat[b,c,q,d]+1.00001)/2*28) == n).

Strategy: pure data parallelism over the batch dim (8 batches/core).
Per core, 8 tiles of [128 rows = (c,q), 4096] are processed.

Binning is via cumulative counts cnt_lt(t) = #unmasked{y < t}, t=1..29,
y = (x + 1.00001) * 14 (two fp32 roundings, bit-identical to the
reference's (x+1.00001)/2*28 since the /2 is exact). hist = adjacent
differences (host side, exact integer arithmetic).

v1: 29 fused DVE scalar_tensor_tensor ops/tile ((y<t)*w + row-sum).
v3: integer bins are computed once via r = int-convert(y) plus the
    rounding-mode-agnostic fixup floor = r - (r > y), then mask-merged
    to bm = (floor-100)*w in bf16 (unmasked: exact integers in
    [-100,-72]; masked: 0, above every shifted threshold t-100).
    The 29 thresholds are split across engines, one fused op each per
    tile (measured ~4.3-4.4us per [128,4096] op on HW; accum_out
    forces 1x rate on DVE, so DVE and ACT ops cost about the same):
      DVE:  tensor_scalar is_lt + accum row-sum      (13 thresholds)
      ACT:  Sign(bm + (100.5-t)) + accum row-sum     (16 thresholds,
            cnt_lt = (N - S)/2, masked elements cancel)
    GPSIMD runs the convert, subtract and mask-merge prep ops.
"""
import numpy as np
from contextlib import ExitStack

import concourse.bass as bass
import concourse.tile as tile
from concourse import bacc, mybir
from concourse.bass_utils import run_bass_kernel_spmd

B, C, Q, D = 64, 4, 32, 4096
NCORES = 8
BPC = B // NCORES          # batches per core
ROWS = C * Q               # 128 rows per batch -> one SBUF tile
NB = 29
NT = 32                    # padded threshold columns in the output

AOT = mybir.AluOpType
AFT = mybir.ActivationFunctionType
F32 = mybir.dt.float32
BF16 = mybir.dt.bfloat16

# thresholds t=1..29; these go to the scalar (ACT) engine, rest to DVE.
# HW measurement: tensor_scalar with accum_out runs at 1x (~4.4us/op on
# [128,4096]), same as one ACT Sign+accum op (~4.25us) -> split nearly
# evenly, ACT slightly favored since DVE also carries the prep ops.
ACT_T = list(range(22, 30))
SPLIT_T = 29               # this ACT threshold's columns are split ACT/DVE
SPLIT_D = 2048             # ACT takes columns [0, SPLIT_D), DVE the rest

_nc_cache = {}
last_results = None  # test.py reads exec info from here
TRACE = False

STRATEGY = "v3"


def _build_v1(nc, x_ap, w_ap, o_ap, c_ap, tc, ctx):
    xp = ctx.enter_context(tc.tile_pool(name="xp", bufs=2))
    wp = ctx.enter_context(tc.tile_pool(name="wp", bufs=2))
    sp = ctx.enter_context(tc.tile_pool(name="sp", bufs=2))
    op = ctx.enter_context(tc.tile_pool(name="op", bufs=2))
    for t in range(BPC):
        xt = xp.tile([ROWS, D], F32)
        nc.sync.dma_start(xt[:], x_ap[t])
        wt = wp.tile([ROWS, D], F32)
        for c in range(C):
            nc.sync.dma_start(wt[c * Q:(c + 1) * Q, :], w_ap[t])
        y = sp.tile([ROWS, D], F32, tag="y")
        nc.vector.tensor_scalar(y[:], xt[:], 1.00001, 14.0, AOT.add, AOT.mult)
        scr = sp.tile([ROWS, D], F32, tag="scr")
        cnt = op.tile([ROWS, NT], F32)
        for n in range(1, NB + 1):
            nc.vector.scalar_tensor_tensor(
                scr[:], y[:], float(n), wt[:], AOT.is_lt, AOT.mult,
                accum_out=cnt[:, n - 1:n])
        nc.sync.dma_start(o_ap[t, :, 0:NT], cnt[:])


def _build_v3(nc, x_ap, w_ap, o_ap, c_ap, tc, ctx):
    dve_t = [t for t in range(1, NB + 1) if t not in ACT_T]
    cp = ctx.enter_context(tc.tile_pool(name="cp", bufs=1))
    bias = cp.tile([ROWS, NT], F32)
    nc.sync.dma_start(bias[:], c_ap[:])

    xp = ctx.enter_context(tc.tile_pool(name="xp", bufs=2))
    wp = ctx.enter_context(tc.tile_pool(name="wp", bufs=2))
    yp = ctx.enter_context(tc.tile_pool(name="yp", bufs=2))
    ip = ctx.enter_context(tc.tile_pool(name="ip", bufs=2))
    bp = ctx.enter_context(tc.tile_pool(name="bp", bufs=2))
    gp = ctx.enter_context(tc.tile_pool(name="gp", bufs=2))
    mp = ctx.enter_context(tc.tile_pool(name="mp", bufs=2))
    sp = ctx.enter_context(tc.tile_pool(name="sp", bufs=1))
    op = ctx.enter_context(tc.tile_pool(name="op", bufs=2))

    for t in range(BPC):
        xt = xp.tile([ROWS, D], F32)
        nc.sync.dma_start(xt[:], x_ap[t])
        wt = wp.tile([ROWS, D], BF16)
        for c in range(C):
            nc.sync.dma_start(wt[c * Q:(c + 1) * Q, :], w_ap[t])

        # y = (x + 1.00001) * 14       [DVE, fp32 2x]
        y = yp.tile([ROWS, D], F32, tag="y")
        nc.vector.tensor_scalar(y[:], xt[:], 1.00001, 14.0, AOT.add, AOT.mult)
        # r = int(y) (i32; RNE on DVE, rounding mode irrelevant: the
        # gt-fixup below corrects any bi in {floor, floor+1})  [GPSIMD]
        bi = ip.tile([ROWS, D], mybir.dt.int32, tag="bi")
        nc.gpsimd.tensor_copy(bi[:], y[:])
        # b100 = r - 100 -> bf16       [GPSIMD, i32 in]
        b100 = bp.tile([ROWS, D], BF16, tag="b100")
        nc.gpsimd.tensor_scalar(b100[:], bi[:], 100.0, None, AOT.subtract)
        # gt = (r > y) -> bf16         [DVE TT 1x]
        gt = gp.tile([ROWS, D], BF16, tag="gt")
        nc.vector.tensor_tensor(gt[:], bi[:], y[:], AOT.is_gt)
        # bm0 = floor(y)-100 = b100-gt [GPSIMD bf16]
        bm0 = mp.tile([ROWS, D], BF16, tag="bm0")
        nc.gpsimd.tensor_tensor(bm0[:], b100[:], gt[:], AOT.subtract)
        # bm = bm0 * w                 [GPSIMD bf16]
        bm = mp.tile([ROWS, D], BF16, tag="bm")
        nc.gpsimd.tensor_tensor(bm[:], bm0[:], wt[:], AOT.mult)

        cnt = op.tile([ROWS, NT], F32, tag="cnt")
        scr_d = sp.tile([ROWS, D], BF16, tag="scr_d")
        scr_a = sp.tile([ROWS, D], BF16, tag="scr_a")
        # interleave DVE and ACT issue so both engines fill early
        order = []
        di, ai = 0, 0
        while di < len(dve_t) or ai < len(ACT_T):
            if di < len(dve_t):
                order.append(("d", dve_t[di])); di += 1
            if ai < len(ACT_T):
                order.append(("a", ACT_T[ai])); ai += 1
        for kind, n in order:
            if kind == "d":
                # cnt[:, n-1] = sum_d (bm < n-100)          [DVE 1x]
                nc.vector.tensor_scalar(scr_d[:], bm[:], float(n - 100), None,
                                        AOT.is_lt, AOT.add,
                                        accum_out=cnt[:, n - 1:n])
            elif n == SPLIT_T:
                # ACT is the critical path by ~1 op-half: split this
                # threshold's columns between ACT (first SPLIT_D) and
                # DVE (rest); host adds the two partial counts.
                nc.scalar.activation(scr_a[:, 0:SPLIT_D], bm[:, 0:SPLIT_D],
                                     AFT.Sign, bias=bias[:, n - 1:n],
                                     accum_out=cnt[:, n - 1:n])
                nc.vector.tensor_scalar(scr_d[:, SPLIT_D:D], bm[:, SPLIT_D:D],
                                        float(n - 100), None,
                                        AOT.is_lt, AOT.add,
                                        accum_out=cnt[:, NB:NB + 1])
            else:
                # cnt[:, n-1] = sum_d sign(bm + (100.5-n))  [ACT]
                nc.scalar.activation(scr_a[:], bm[:], AFT.Sign,
                                     bias=bias[:, n - 1:n],
                                     accum_out=cnt[:, n - 1:n])
        nc.sync.dma_start(o_ap[t, :, 0:NT], cnt[:])


def _build(strategy=None):
    strategy = strategy or STRATEGY
    if strategy in _nc_cache:
        return _nc_cache[strategy]
    nc = bacc.Bacc("TRN2", target_bir_lowering=False, debug=False,
                   enable_asserts=False, num_devices=NCORES)
    x_ap = nc.dram_tensor("x", [BPC, ROWS, D], F32, kind="ExternalInput").ap()
    w_dt = F32 if strategy == "v1" else BF16
    w_ap = nc.dram_tensor("w", [BPC, Q, D], w_dt, kind="ExternalInput").ap()
    c_ap = nc.dram_tensor("consts", [ROWS, NT], F32, kind="ExternalInput").ap()
    o_ap = nc.dram_tensor("o", [BPC, ROWS, NT], F32, kind="ExternalOutput").ap()

    with tile.TileContext(nc) as tc, ExitStack() as ctx:
        if strategy == "v1":
            _build_v1(nc, x_ap, w_ap, o_ap, c_ap, tc, ctx)
        elif strategy == "v3":
            _build_v3(nc, x_ap, w_ap, o_ap, c_ap, tc, ctx)
        else:
            raise ValueError(strategy)
    nc.compile()
    _nc_cache[strategy] = nc
    return nc


def _make_consts():
    consts = np.zeros((ROWS, NT), np.float32)
    for n in ACT_T:
        consts[:, n - 1] = 100.5 - n
    return consts


def kernel(simmat: np.ndarray, mask: np.ndarray) -> np.ndarray:
    global last_results
    strategy = STRATEGY
    nc = _build(strategy)
    consts = _make_consts()
    in_maps = []
    import ml_dtypes
    w_np = np.float32 if strategy == "v1" else ml_dtypes.bfloat16
    for ci in range(NCORES):
        sl = slice(ci * BPC, (ci + 1) * BPC)
        xs = np.ascontiguousarray(
            np.asarray(simmat[sl], dtype=np.float32).reshape(BPC, ROWS, D))
        ws = np.ascontiguousarray(np.asarray(mask[sl]).astype(w_np))
        in_maps.append({"x": xs, "w": ws, "consts": consts})
    res = run_bass_kernel_spmd(nc, in_maps, core_ids=list(range(NCORES)),
                               trace=TRACE)
    last_results = res
    raw = np.concatenate([r["o"] for r in res.results], axis=0)
    raw = raw.reshape(B * C * Q, NT)

    # reconstruct cnt_lt(t), t=1..29 per row
    cnt = np.empty((B * C * Q, NB), np.float64)
    if strategy == "v1":
        cnt[:] = raw[:, :NB]
    else:
        for t in range(1, NB + 1):
            if t == SPLIT_T:
                cnt[:, t - 1] = ((SPLIT_D - raw[:, t - 1]) * 0.5
                                 + raw[:, NB])
            elif t in ACT_T:
                cnt[:, t - 1] = (D - raw[:, t - 1]) * 0.5
            else:
                cnt[:, t - 1] = raw[:, t - 1]
    hist = np.diff(cnt, prepend=0.0, axis=-1)
    return np.ascontiguousarray(
        hist.reshape(B, C, Q, NB).astype(np.float32))

